# revision 14
# baseline (speedup 1.0000x reference)
"""Bahdanau attention Trainium2 kernel.

Data-parallel over batch: B=32 examples -> 8 NeuronCores x 4 examples.
Per example (T=4096, D=U=256):
  projT[u,t]  = sum_d W1[d,u] * enc[t,d]          (PE, W1 stationary, bf16)
  tanhT[u,t]  = tanh(projT + (hidden@W2 + W1_b + W2_b)[u])   (ACT, per-partition bias)
  score[t]    = sum_u V[u] * tanhT[u,t]           (PE, M=1)
  attn        = softmax(score)   (no max-subtraction: |score| <= 16 and exp is fp32)
  context[d]  = sum_t attn[t] * enc[t,d]          (PE, attn chunks stationary)

Both layouts of enc (D-major for the projection, T-major for the context
matmul) are prepared host-side in bf16 so every DMA is contiguous. The
score row is written in a permuted order so one strided DMA redistributes
it from one partition to [128, 32]; a DVE 32x32 block-transpose then yields
matmul-ready attn chunks. The context matmul uses unnormalized exp-weights;
1/sum is folded into the final context copy.
"""

import numpy as np
import ml_dtypes

B, T, D, U = 32, 4096, 256, 256
N_CORES = 8
EPC = B // N_CORES  # examples per core

# packed weight layouts (columns)
_BF_COLS = 2 * U + 2          # W1 k-chunks [256+256], V chunks [1+1]
_F32_COLS = 4 * 128 + 2 * EPC + 2  # W2 chunks, hidT chunks, bias chunks

_cache = {}


def _build(n_reps: int = 1):
    """Build + compile the per-core Bass program."""
    from contextlib import ExitStack

    import concourse.bass as bass
    import concourse.bass_isa as bass_isa
    import concourse.tile as tile
    from concourse import bacc, mybir

    f32 = mybir.dt.float32
    bf16 = mybir.dt.bfloat16
    AF = mybir.ActivationFunctionType

    nc = bacc.Bacc("TRN2", target_bir_lowering=False, debug=False, num_devices=1)

    encT_d = nc.dram_tensor("encT", [EPC, 2, 128, T], bf16, kind="ExternalInput").ap()
    encN_d = nc.dram_tensor("encN", [EPC, 128, 32, D], bf16, kind="ExternalInput").ap()
    wbf_d = nc.dram_tensor("wbf", [128, _BF_COLS], bf16, kind="ExternalInput").ap()
    wf_d = nc.dram_tensor("wf", [128, _F32_COLS], f32, kind="ExternalInput").ap()

    attn_d = nc.dram_tensor("attn", [EPC, T], f32, kind="ExternalOutput").ap()
    ctx_d = nc.dram_tensor("ctx", [EPC, D], f32, kind="ExternalOutput").ap()

    with tile.TileContext(nc) as tc, ExitStack() as ctx:
        wpool = ctx.enter_context(tc.tile_pool(name="w", bufs=1))
        encT_pool = ctx.enter_context(tc.tile_pool(name="encT", bufs=2))
        encN_pool = ctx.enter_context(tc.tile_pool(name="encN", bufs=2))
        tanh_pool = ctx.enter_context(tc.tile_pool(name="tanh", bufs=4))
        small = ctx.enter_context(tc.tile_pool(name="small", bufs=2))
        psA = ctx.enter_context(tc.tile_pool(name="psA", bufs=2, space="PSUM"))
        psS = ctx.enter_context(tc.tile_pool(name="psS", bufs=4, space="PSUM"))

        # --- weights: two packed DMAs ---
        wbf = wpool.tile([128, _BF_COLS], bf16, tag="wbf")
        nc.sync.dma_start(out=wbf, in_=wbf_d)
        wf = wpool.tile([128, _F32_COLS], f32, tag="wf")
        nc.sync.dma_start(out=wf, in_=wf_d)

        W1_sb = [wbf[:, bass.ts(k, U)] for k in range(2)]
        V_sb = [wbf[:, 2 * U + k : 2 * U + k + 1] for k in range(2)]
        W2_sb = [
            [wf[:, bass.ts(2 * k + m, 128)] for m in range(2)] for k in range(2)
        ]
        hid_sb = [
            wf[:, 512 + k * EPC : 512 + (k + 1) * EPC] for k in range(2)
        ]
        bias_sb = [
            wf[:, 512 + 2 * EPC + m : 512 + 2 * EPC + m + 1] for m in range(2)
        ]

        # --- hidden projection: projhid[m][u, e] = (hidden @ W2 + biases)[e, m*128+u]
        projhid = []
        for m in range(2):
            ph_ps = psS.tile([128, EPC], f32, tag="sp")
            nc.tensor.matmul(ph_ps, W2_sb[0][m], hid_sb[0], start=True, stop=False)
            nc.tensor.matmul(ph_ps, W2_sb[1][m], hid_sb[1], start=False, stop=True)
            ph = wpool.tile([128, EPC], f32, tag=f"ph_{m}")
            nc.scalar.activation(ph, ph_ps, AF.Identity, bias=bias_sb[m], scale=1.0)
            projhid.append(ph)

        for rep in range(n_reps):
            loads = {}

            def issue_loads(e):
                if e >= EPC:
                    return
                t_ = encT_pool.tile([128, 2, T], bf16, tag="encT")
                nc.sync.dma_start(
                    out=t_, in_=encT_d[e].rearrange("k d t -> d k t")
                )
                n_ = encN_pool.tile([128, 32, D], bf16, tag="encN")
                nc.sync.dma_start(out=n_, in_=encN_d[e])
                loads[e] = (t_, n_)

            issue_loads(0)
            issue_loads(1)
            for e in range(EPC):
                encT_sb, encN_sb = loads.pop(e)

                # projection + tanh, batched into [128, 1024] psum groups so
                # each ACT op amortizes its ~352-cycle fixed cost; LDW of the
                # same W1 chunk covers two matmuls.
                tanh_sb = []
                for m in range(2):
                    th = tanh_pool.tile([128, T], bf16, tag="tanh")
                    tanh_sb.append(th)
                    for g in range(4):
                        pp = psA.tile([128, 1024], f32)
                        for k in range(2):
                            for h in range(2):
                                nc.tensor.matmul(
                                    pp[:, bass.ts(h, 512)],
                                    W1_sb[k][:, bass.ts(m, 128)],
                                    encT_sb[:, k, bass.ts(g * 2 + h, 512)],
                                    start=(k == 0), stop=(k == 1),
                                )
                        nc.scalar.activation(
                            th[:, bass.ts(g, 1024)], pp, AF.Tanh,
                            bias=projhid[m][:, e : e + 1], scale=1.0,
                        )

                # V projection -> permuted score row.
                # score_perm[sigma(t)] = score[t], sigma(128b+32a+c) = 1024a+32b+c,
                # so the redistribution below is one contiguous-dest DMA.
                score = small.tile([1, T], f32, tag="score")
                for t8 in range(8):
                    sl = bass.ts(t8, 512)
                    ps = psS.tile([1, 512], f32, tag="sp")
                    nc.tensor.matmul(ps, V_sb[0], tanh_sb[0][:, sl], start=True, stop=False)
                    nc.tensor.matmul(ps, V_sb[1], tanh_sb[1][:, sl], start=False, stop=True)
                    # chunk t8 holds t = 512*t8 + (128 b' + 32 a' + c);
                    # sigma(t) = 1024 a' + 128 t8 + 32 b' + c
                    out_v = bass.AP(
                        tensor=score.tensor,
                        offset=score.offset + 128 * t8,
                        ap=[[score.ap[0][0], 1], [32, 4], [1024, 4], [1, 32]],
                    )
                    nc.vector.tensor_copy(out=out_v, in_=ps.rearrange(
                        "o (b a c) -> o b a c", b=4, a=4, c=32
                    ))

                # one-DMA redistribution: C[p, c] = score_perm[32 p + c]
                C = small.tile([128, 32], f32, tag="C")
                nc.sync.dma_start(
                    out=C, in_=score.rearrange("o (p c) -> o p c", p=128, c=32)
                )
                E = small.tile([128, 32], f32, tag="E")
                S1 = small.tile([128, 1], f32, tag="S1")
                nc.scalar.activation(E, C, AF.Exp, accum_out=S1)
                # context matmul uses unnormalized exp-weights; 1/sum is folded
                # into the final context copy, so the PE chain only waits on
                # the transpose below, not on the reduction.
                Eb = small.tile([128, 32], bf16, tag="Eb")
                nc.vector.tensor_copy(out=Eb, in_=E)
                EB = small.tile([128, 32], bf16, tag="EB")
                nc.vector.transpose(EB, Eb)
                tot = small.tile([128, 1], f32, tag="tot")
                nc.gpsimd.partition_all_reduce(
                    tot, S1, channels=128, reduce_op=bass_isa.ReduceOp.add
                )
                inv = small.tile([128, 1], f32, tag="inv")
                nc.vector.reciprocal(inv, tot)
                attnC = small.tile([128, 32], f32, tag="attnC")
                nc.vector.tensor_scalar_mul(attnC, E, inv)
                nc.sync.dma_start(
                    out=attn_d[e].rearrange("(b a c) -> a b c", b=32, a=4, c=32),
                    in_=attnC,
                )

                # context: sum_t exp[t] * enc[t, :], scaled by 1/sum at the end
                ctx_ps = psS.tile([1, D], f32, tag="sp")
                for c in range(32):
                    nc.tensor.matmul(
                        ctx_ps, EB[:, c : c + 1], encN_sb[:, c, :],
                        start=(c == 0), stop=(c == 31),
                    )
                ctx_sb = small.tile([1, D], f32, tag="ctx")
                nc.scalar.mul(ctx_sb, ctx_ps, inv[0:1, :])
                nc.sync.dma_start(out=ctx_d[e], in_=ctx_sb)

                issue_loads(e + 2)

    nc.compile()
    return nc


def _prep_inputs(encoder_out, hidden, W1_w, W1_b, W2_w, W2_b, V_w, V_b):
    bf16 = ml_dtypes.bfloat16
    enc = np.ascontiguousarray(encoder_out, dtype=np.float32)

    wbf = np.zeros((128, _BF_COLS), dtype=bf16)
    wbf[:, 0:U] = W1_w[0:128, :].astype(bf16)
    wbf[:, U : 2 * U] = W1_w[128:256, :].astype(bf16)
    wbf[:, 2 * U : 2 * U + 1] = V_w[0:128, :].astype(bf16)
    wbf[:, 2 * U + 1 : 2 * U + 2] = V_w[128:256, :].astype(bf16)

    bias = (W1_b + W2_b).astype(np.float32)

    in_maps = []
    for core in range(N_CORES):
        sh = enc[core * EPC : (core + 1) * EPC]  # [EPC, T, D]
        encT = (
            np.ascontiguousarray(sh.transpose(0, 2, 1))
            .reshape(EPC, 2, 128, T)
            .astype(bf16)
        )
        encN = np.ascontiguousarray(
            sh.reshape(EPC, 32, 128, D).transpose(0, 2, 1, 3)
        ).astype(bf16)

        hid = hidden[core * EPC : (core + 1) * EPC]  # [EPC, D]
        wf = np.zeros((128, _F32_COLS), dtype=np.float32)
        for k in range(2):
            for m in range(2):
                wf[:, (2 * k + m) * 128 : (2 * k + m + 1) * 128] = W2_w[
                    k * 128 : (k + 1) * 128, m * 128 : (m + 1) * 128
                ]
            wf[:, 512 + k * EPC : 512 + (k + 1) * EPC] = hid[
                :, k * 128 : (k + 1) * 128
            ].T
        wf[:, 512 + 2 * EPC + 0] = bias[0:128]
        wf[:, 512 + 2 * EPC + 1] = bias[128:256]

        in_maps.append({"encT": encT, "encN": encN, "wbf": wbf, "wf": wf})
    return in_maps


def run_on_cores(in_maps, n_reps: int = 1):
    from concourse.bass_utils import run_bass_kernel_spmd

    if n_reps not in _cache:
        _cache[n_reps] = _build(n_reps)
    nc = _cache[n_reps]
    return run_bass_kernel_spmd(nc, in_maps, core_ids=list(range(N_CORES)))


def kernel(encoder_out, hidden, W1_w, W1_b, W2_w, W2_b, V_w, V_b):
    in_maps = _prep_inputs(
        encoder_out, hidden, W1_w, W1_b, W2_w, W2_b, V_w, V_b
    )
    res = run_on_cores(in_maps)
    attn = np.concatenate([res.results[c]["attn"] for c in range(N_CORES)], axis=0)
    context = np.concatenate([res.results[c]["ctx"] for c in range(N_CORES)], axis=0)
    return (
        context.astype(np.float32),
        attn.reshape(B, T, 1).astype(np.float32),
    )
